# revision 25
# baseline (speedup 1.0000x reference)
"""Trainium2 Bass kernel for nn_BilinearAttention.

Reference computation (per batch element b, fully independent):
    YVa = relu(Y[b] @ V_attn^T)          # [M, K]
    YVj = relu(Y[b] @ V_joint^T)         # [M, K]
    f = X[b]
    for g in range(4):
        w      = relu(U_attn @ f) * p[g]          # [K]
        logits = YVa @ w                          # [M]
        attn   = softmax(logits)                  # [M]
        vy     = attn @ YVj                       # [K]
        z      = relu(U_joint @ f) * vy           # [K]
        f      = layernorm(f + P @ z) * gamma[g] + beta[g]

Sharding: data-parallel over bs=32 across 8 cores (4 batches/core), weights
replicated, no collectives.  Per core the 4 batches run in 2 groups of 2:
YVa^T (k-major, bf16) stays SBUF-resident per group, YVj (m-major, bf16) is
spilled to DRAM scratch and streamed back during the attention-weighted sum.
"""

import sys

import numpy as np

sys.path.insert(0, "/opt/trn_rl_repo")

import concourse.bass as bass
import concourse.bacc as bacc
import concourse.tile as tile
from concourse import mybir
from concourse.masks import make_identity

# Problem constants (hardcoded per contract).
BS = 32
M = 2048
D = 1024  # d_q == d_t
K = 1024
G = 4
EPS = 1e-5
N_CORES = 8
NB = BS // N_CORES  # local batches per core = 4
GRP = 2             # batches per resident group
P = 128             # partitions
FP32 = mybir.dt.float32
BF16 = mybir.dt.bfloat16
AF = mybir.ActivationFunctionType
ALU = mybir.AluOpType

KT = K // P   # 8 k chunks
DT = D // P   # 8 d chunks
TT = D // P   # 8 t chunks
MT = M // P   # 16 m chunks
MQ = 4        # m quarters of 512
QW = M // MQ  # 512
W_NAMES = ("va", "vj", "ua", "uj", "pm")

# v2: build Y^T via xbar DMA-transpose of a bf16 copy of Y (frees PE/ACT
# from 512 transposes + evictions per core) instead of PE transposes.
USE_DMA_TRANSPOSE = True


def build_core_program():
    """Build the single-core Bass program (same program runs on all 8 cores)."""
    nc = bacc.Bacc("TRN2", target_bir_lowering=False, debug=False,
                   num_devices=N_CORES)

    # ---- DRAM I/O ----
    X = nc.dram_tensor("x", [NB, D], FP32, kind="ExternalInput")
    Y = nc.dram_tensor("y", [NB, M, D], FP32, kind="ExternalInput")
    Ua = nc.dram_tensor("u_attn", [K, D], FP32, kind="ExternalInput")
    Va = nc.dram_tensor("v_attn", [K, D], FP32, kind="ExternalInput")
    Pg = nc.dram_tensor("p_g", [G, K], FP32, kind="ExternalInput")
    Uj = nc.dram_tensor("u_joint", [K, D], FP32, kind="ExternalInput")
    Vj = nc.dram_tensor("v_joint", [K, D], FP32, kind="ExternalInput")
    Pm = nc.dram_tensor("p_mat", [D, K], FP32, kind="ExternalInput")
    Gam = nc.dram_tensor("ln_gamma", [G, D], FP32, kind="ExternalInput")
    Bet = nc.dram_tensor("ln_beta", [G, D], FP32, kind="ExternalInput")

    F_out = nc.dram_tensor("f_out", [NB, D], FP32, kind="ExternalOutput")
    A_out = nc.dram_tensor("attn_out", [G, NB, M], FP32, kind="ExternalOutput")

    # DRAM scratch: transposed bf16 weights + spilled YVj for one group.
    Wscr = nc.dram_tensor("w_scratch", [len(W_NAMES), P, KT, K], BF16)
    Jscr = nc.dram_tensor("yvj_scratch", [GRP, P, MT, K], BF16)
    Ybf = (nc.dram_tensor("y_bf16", [NB, M, D], BF16)
           if USE_DMA_TRANSPOSE else None)

    with tile.TileContext(nc) as tc:
        _emit(nc, tc, X, Y, Ua, Va, Pg, Uj, Vj, Pm, Gam, Bet,
              F_out, A_out, Wscr, Jscr, Ybf)
    nc.compile()
    return nc


def _emit(nc, tc, X, Y, Ua, Va, Pg, Uj, Vj, Pm, Gam, Bet,
          F_out, A_out, Wscr, Jscr, Ybf=None):
    # ----- persistent state -----
    with tc.tile_pool(name="persist", bufs=1) as persist:
        ident = persist.tile([P, P], FP32)
        make_identity(nc, ident[:])

        eps_t = persist.tile([NB, 1], FP32)
        nc.vector.memset(eps_t[:], EPS)
        ones_c = persist.tile([P, 1], FP32)
        nc.vector.memset(ones_c[:], 1.0)
        ones_r = persist.tile([1, P], FP32)
        nc.vector.memset(ones_r[:], 1.0)

        # YVa^T for one group, k on partitions: yva[bl][p, kt, m]
        yva = [persist.tile([P, KT, M], BF16, tag=f"yva{bl}",
                            name=f"yva{bl}")
               for bl in range(GRP)]

        # ---------------- phase 0: weight prep ----------------
        # Transpose+cast all five 1024x1024 weights into DRAM scratch as
        # WT[p, cc, r] = W[r, cc*128+p]  (contraction dim on partitions).
        with (
            tc.tile_pool(name="wload", bufs=3) as wload,
            tc.tile_pool(name="wt_st", bufs=2) as wt_st,
            tc.tile_pool(name="wpsum", bufs=4, space="PSUM") as wpsum,
        ):
            for wi, w_dram in enumerate((Va, Vj, Ua, Uj, Pm)):
                for rc in range(KT):
                    wrow = wload.tile([P, K], FP32, tag="wrow")
                    nc.sync.dma_start(
                        out=wrow[:], in_=w_dram[rc * P:(rc + 1) * P, :]
                    )
                    stg = wt_st.tile([P, KT, P], BF16, tag="wstg")
                    for cc in range(KT):
                        pt = wpsum.tile([P, P], FP32, tag="wt_ps")
                        nc.tensor.transpose(
                            pt[:], wrow[:, cc * P:(cc + 1) * P], ident[:]
                        )
                        nc.scalar.activation(stg[:, cc, :], pt[:], AF.Copy)
                    nc.sync.dma_start(
                        out=Wscr[wi, :, :, rc * P:(rc + 1) * P], in_=stg[:]
                    )

        # ---------------- per-group build + glimpse ----------------
        for grp in range(NB // GRP):
            # ===== build phase =====
            with (
                tc.tile_pool(name="bweights", bufs=1) as bw,
                tc.tile_pool(name="ynat", bufs=3) as ynat,
                tc.tile_pool(name="ytq", bufs=2) as ytq,
                tc.tile_pool(name="jstage", bufs=3) as jstage,
                tc.tile_pool(name="bpsum", bufs=2, space="PSUM") as bpsum,
                tc.tile_pool(name="jpsum", bufs=2, space="PSUM") as jpsum,
            ):
                VaT = bw.tile([P, KT, K], BF16, tag="vat")
                VjT = bw.tile([P, KT, K], BF16, tag="vjt")
                nc.sync.dma_start(out=VaT[:], in_=Wscr[0])
                nc.sync.dma_start(out=VjT[:], in_=Wscr[1])

                for bl in range(GRP):
                    b = grp * GRP + bl
                    if USE_DMA_TRANSPOSE:
                        # bf16 copy of Y[b] (cast during DMA), then xbar
                        # DMA-transpose straight into SBUF: ytf[t, m].
                        for mc in range(MT):
                            cb = ynat.tile([P, D], BF16, tag="ycast")
                            nc.gpsimd.dma_start(
                                out=cb[:], in_=Y[b, mc * P:(mc + 1) * P, :]
                            )
                            nc.sync.dma_start(
                                out=Ybf[b, mc * P:(mc + 1) * P, :], in_=cb[:]
                            )
                        ytf = ytq.tile([P, TT, M], BF16, tag="ytf")
                        for t_ in range(TT):
                            nc.sync.dma_start_transpose(
                                out=ytf[:, t_, :],
                                in_=Ybf[b, :, t_ * P:(t_ + 1) * P],
                            )
                        quarters = [(ytf, q) for q in range(MQ)]
                    else:
                        quarters = None

                    for q in range(MQ):
                        if USE_DMA_TRANSPOSE:
                            yt = ytf
                            moff = q * QW      # m offset inside yt tiles
                        else:
                            # Load + PE-transpose Y[b, q*512:(q+1)*512, :]
                            yt = ytq.tile([P, TT, QW], BF16, tag="ytq")
                            moff = 0
                            for mc in range(QW // P):
                                mg = q * (QW // P) + mc
                                yn = ynat.tile([P, D], FP32, tag="ynat")
                                nc.sync.dma_start(
                                    out=yn[:], in_=Y[b, mg * P:(mg + 1) * P, :]
                                )
                                for t_ in range(TT):
                                    pt = bpsum.tile([P, P], FP32, tag="yt_ps")
                                    nc.tensor.transpose(
                                        pt[:], yn[:, t_ * P:(t_ + 1) * P],
                                        ident[:],
                                    )
                                    nc.scalar.activation(
                                        yt[:, t_, mc * P:(mc + 1) * P],
                                        pt[:], AF.Copy,
                                    )

                        # YVa^T[k, m-quarter]: accumulate over t.
                        for kt in range(KT):
                            ps = bpsum.tile([P, QW], FP32, tag="yva_ps")
                            for t_ in range(TT):
                                nc.tensor.matmul(
                                    ps[:],
                                    VaT[:, t_, kt * P:(kt + 1) * P],
                                    yt[:, t_, moff:moff + QW],
                                    start=(t_ == 0),
                                    stop=(t_ == TT - 1),
                                )
                            nc.scalar.activation(
                                yva[bl][:, kt, q * QW:(q + 1) * QW],
                                ps[:], AF.Relu,
                            )

                        # YVj[m-chunk, k] -> DRAM scratch, acc over t.
                        for mc in range(QW // P):
                            mg = q * (QW // P) + mc
                            ps = jpsum.tile([P, K], FP32, tag="yvj_ps")
                            for t_ in range(TT):
                                for kh in range(2):
                                    nc.tensor.matmul(
                                        ps[:, kh * QW:(kh + 1) * QW],
                                        yt[:, t_,
                                           moff + mc * P:moff + (mc + 1) * P],
                                        VjT[:, t_, kh * QW:(kh + 1) * QW],
                                        start=(t_ == 0),
                                        stop=(t_ == TT - 1),
                                    )
                            stg = jstage.tile([P, K], BF16, tag="jstg")
                            for kh in range(2):
                                nc.scalar.activation(
                                    stg[:, kh * QW:(kh + 1) * QW],
                                    ps[:, kh * QW:(kh + 1) * QW],
                                    AF.Relu,
                                )
                            nc.sync.dma_start(
                                out=Jscr[bl, :, mg, :], in_=stg[:]
                            )

            # ===== glimpse phase =====
            rows = slice(grp * GRP, grp * GRP + GRP)
            with (
                tc.tile_pool(name="gweights", bufs=1) as gw,
                tc.tile_pool(name="gvec", bufs=1) as gvec,
                tc.tile_pool(name="jstream", bufs=2) as jstream,
                tc.tile_pool(name="gsmall", bufs=2) as gs,
                tc.tile_pool(name="gcols", bufs=2) as gcols,
                tc.tile_pool(name="gl_psum", bufs=1, space="PSUM") as glp,
            ):
                UaT = gw.tile([P, KT, K], BF16, tag="uat")
                UjT = gw.tile([P, KT, K], BF16, tag="ujt")
                PT = gw.tile([P, KT, K], BF16, tag="pt")
                nc.sync.dma_start(out=UaT[:], in_=Wscr[2])
                nc.sync.dma_start(out=UjT[:], in_=Wscr[3])
                nc.sync.dma_start(out=PT[:], in_=Wscr[4])

                # f rows for this group live at partitions [0, GRP)
                fgrp = gvec.tile([GRP, D], FP32, tag="fgrp")
                nc.sync.dma_start(out=fgrp[:], in_=X[rows, :])

                for g in range(G):
                    # Per-glimpse row vectors replicated across GRP rows.
                    gamv = gvec.tile([GRP, D], FP32, tag="gamv", bufs=2)
                    betv = gvec.tile([GRP, D], FP32, tag="betv", bufs=2)
                    nc.gpsimd.dma_start(
                        out=gamv[:], in_=Gam[g:g + 1, :].to_broadcast([GRP, D])
                    )
                    nc.gpsimd.dma_start(
                        out=betv[:], in_=Bet[g:g + 1, :].to_broadcast([GRP, D])
                    )
                    # p[g] in column layout, replicated over GRP columns
                    pcol = gvec.tile([P, KT, GRP], BF16, tag="pcol", bufs=2)
                    for bl in range(GRP):
                        nc.gpsimd.dma_start(
                            out=pcol[:, :, bl],
                            in_=Pg[g, :].rearrange("(c p) -> p c", p=P),
                        )

                    # f columns bf16 via PE transpose: fT[p, dc, bl]
                    ftp = glp.tile([P, DT, GRP], FP32, tag="col_ps")
                    for dc in range(DT):
                        nc.tensor.transpose(
                            ftp[:, dc, :],
                            fgrp[:, dc * P:(dc + 1) * P],
                            ident[0:GRP, 0:GRP],
                        )
                    fT = gcols.tile([P, DT, GRP], BF16, tag="fT")
                    nc.scalar.activation(fT[:], ftp[:], AF.Copy)

                    # XU^T columns directly: xuT[p, kt, bl] = relu(U @ f)
                    xuaT = gcols.tile([P, KT, GRP], BF16, tag="xuaT")
                    xujT = gcols.tile([P, KT, GRP], BF16, tag="xujT")
                    for dst, wt in ((xuaT, UaT), (xujT, UjT)):
                        ps = glp.tile([P, KT, GRP], FP32, tag="col_ps")
                        for kt in range(KT):
                            for dc in range(DT):
                                nc.tensor.matmul(
                                    ps[:, kt, :],
                                    wt[:, dc, kt * P:(kt + 1) * P],
                                    fT[:, dc, :],
                                    start=(dc == 0),
                                    stop=(dc == DT - 1),
                                )
                        nc.scalar.activation(dst[:], ps[:], AF.Relu)

                    # w columns = xuaT * p[g]
                    wT = gcols.tile([P, KT, GRP], BF16, tag="wT")
                    nc.vector.tensor_mul(wT[:], xuaT[:], pcol[:])

                    vyT = gcols.tile([P, KT, GRP], BF16, tag="vyT")
                    for bl in range(GRP):
                        b = grp * GRP + bl
                        # logits in m-partition layout: lg[p, mt],
                        # m = mt*128 + p.  yva blocks are the weights.
                        lps = glp.tile([P, MT], FP32, tag="log_ps", bufs=2)
                        for mt in range(MT):
                            for kt in range(KT):
                                nc.tensor.matmul(
                                    lps[:, mt:mt + 1],
                                    yva[bl][:, kt, mt * P:(mt + 1) * P],
                                    wT[:, kt, bl:bl + 1],
                                    start=(kt == 0),
                                    stop=(kt == KT - 1),
                                )
                        # softmax (logits are O(1): skip max subtraction)
                        expv = gs.tile([P, MT], FP32, tag="expv")
                        erow = gs.tile([P, 1], FP32, tag="erow")
                        nc.scalar.activation(
                            expv[:], lps[:], AF.Exp, accum_out=erow[:]
                        )
                        # total = ones . erow (cross-partition sum on PE)
                        tps = glp.tile([1, 1], FP32, tag="sm_ps", bufs=2)
                        nc.tensor.matmul(
                            tps[:], ones_c[:], erow[:], start=True, stop=True
                        )
                        tsum = gs.tile([1, 1], FP32, tag="tsum")
                        nc.scalar.activation(tsum[:], tps[:], AF.Copy)
                        rsum = gs.tile([1, 1], FP32, tag="rsum")
                        nc.vector.reciprocal(rsum[:], tsum[:])
                        # broadcast 1/total to all partitions via PE
                        bps = glp.tile([P, 1], FP32, tag="sm_ps", bufs=2)
                        nc.tensor.matmul(
                            bps[:], ones_r[:], rsum[:], start=True, stop=True
                        )
                        rcol = gs.tile([P, 1], FP32, tag="rcol")
                        nc.scalar.activation(rcol[:], bps[:], AF.Copy)
                        # attn = expv * rcol: fp32 copy for output DMA,
                        # bf16 copy as Vy matmul weights
                        attn_f = gs.tile([P, MT], FP32, tag="attn_f")
                        nc.vector.tensor_scalar_mul(attn_f[:], expv[:], rcol[:])
                        attnT = gs.tile([P, MT], BF16, tag="attnT")
                        nc.vector.tensor_scalar_mul(attnT[:], expv[:], rcol[:])
                        nc.gpsimd.dma_start(
                            out=A_out[g, b, :].rearrange("(c p) -> p c", p=P),
                            in_=attn_f[:],
                        )

                        # vy[k] = attn . YVj[m, k], streaming YVj from DRAM
                        vps = glp.tile([1, K], FP32, tag="kd_ps")
                        NMC = 4  # m-chunks per streamed tile
                        for mt0 in range(0, MT, NMC):
                            yjs = jstream.tile([P, NMC, K], BF16, tag="yjs")
                            nc.sync.dma_start(
                                out=yjs[:],
                                in_=Jscr[bl, :, mt0:mt0 + NMC, :],
                            )
                            for mi in range(NMC):
                                mt = mt0 + mi
                                for kh in range(2):
                                    nc.tensor.matmul(
                                        vps[:, kh * QW:(kh + 1) * QW],
                                        attnT[:, mt:mt + 1],
                                        yjs[:, mi, kh * QW:(kh + 1) * QW],
                                        start=(mt == 0),
                                        stop=(mt == MT - 1),
                                    )
                        vyrow = gs.tile([1, K], FP32, tag="vyrow")
                        nc.scalar.activation(vyrow[:], vps[:], AF.Copy)
                        nc.gpsimd.dma_start(
                            out=vyT[:, :, bl],
                            in_=vyrow[0, :].rearrange("(c p) -> p c", p=P),
                        )

                    # z = xuj * vy (columns, bf16), f_joint = P @ z
                    zT = gcols.tile([P, KT, GRP], BF16, tag="zT")
                    nc.vector.tensor_mul(zT[:], xujT[:], vyT[:])

                    fj = glp.tile([GRP, D], FP32, tag="kd_ps")
                    for dh in range(2):
                        for kc in range(KT):
                            nc.tensor.matmul(
                                fj[:, dh * QW:(dh + 1) * QW],
                                zT[:, kc, :],
                                PT[:, kc, dh * QW:(dh + 1) * QW],
                                start=(kc == 0),
                                stop=(kc == KT - 1),
                            )

                    # f = layernorm(f + fj) * gamma[g] + beta[g]
                    fnew = gs.tile([GRP, D], FP32, tag="fnew")
                    nc.vector.tensor_add(fnew[:], fgrp[:], fj[:])
                    stats = gs.tile([GRP, 2, 6], FP32, tag="stats")
                    for sg in range(2):
                        nc.vector.bn_stats(
                            out=stats[:, sg, :],
                            in_=fnew[:, sg * QW:(sg + 1) * QW],
                        )
                    mv = gs.tile([GRP, 2], FP32, tag="mv")
                    nc.vector.bn_aggr(out=mv[:], in_=stats[:])
                    # rstd = exp(-0.5*ln(var+eps)) (stays in exp/ln table set)
                    lnv = gs.tile([GRP, 1], FP32, tag="lnv")
                    nc.scalar.activation(
                        lnv[:], mv[:, 1:2], AF.Ln, bias=eps_t[0:GRP, :]
                    )
                    rstd = gs.tile([GRP, 1], FP32, tag="rstd")
                    nc.scalar.activation(rstd[:], lnv[:], AF.Exp, scale=-0.5)
                    nc.vector.tensor_scalar(
                        out=fnew[:],
                        in0=fnew[:],
                        scalar1=mv[:, 0:1],
                        scalar2=rstd[:],
                        op0=ALU.subtract,
                        op1=ALU.mult,
                    )
                    nc.vector.tensor_mul(fnew[:], fnew[:], gamv[:])
                    nc.vector.tensor_add(fgrp[:], fnew[:], betv[:])

                # store this group's final f rows
                nc.sync.dma_start(out=F_out[rows, :], in_=fgrp[:])




# ---------------------------------------------------------------------------
# Host-side entry point: full inputs -> full outputs on 8 cores.
# ---------------------------------------------------------------------------
_NC_CACHE = {}


def _get_program():
    if "nc" not in _NC_CACHE:
        _NC_CACHE["nc"] = build_core_program()
    return _NC_CACHE["nc"]


def make_in_maps(X, Y, U_attn, V_attn, p, U_joint, V_joint, P_, ln_gamma,
                 ln_beta):
    in_maps = []
    for c in range(N_CORES):
        sl = slice(c * NB, (c + 1) * NB)
        in_maps.append({
            "x": np.ascontiguousarray(np.asarray(X, np.float32)[sl]),
            "y": np.ascontiguousarray(np.asarray(Y, np.float32)[sl]),
            "u_attn": np.asarray(U_attn, np.float32),
            "v_attn": np.asarray(V_attn, np.float32),
            "p_g": np.asarray(p, np.float32),
            "u_joint": np.asarray(U_joint, np.float32),
            "v_joint": np.asarray(V_joint, np.float32),
            "p_mat": np.asarray(P_, np.float32),
            "ln_gamma": np.asarray(ln_gamma, np.float32),
            "ln_beta": np.asarray(ln_beta, np.float32),
        })
    return in_maps


def kernel(X, Y, U_attn, V_attn, p, U_joint, V_joint, P, ln_gamma, ln_beta):
    from concourse.bass_utils import run_bass_kernel_spmd

    nc = _get_program()
    core_ids = list(range(N_CORES))
    in_maps = make_in_maps(X, Y, U_attn, V_attn, p, U_joint, V_joint, P,
                           ln_gamma, ln_beta)
    res = run_bass_kernel_spmd(nc, in_maps, core_ids)
    f_full = np.concatenate([r["f_out"] for r in res.results], axis=0)
    a_full = np.concatenate([r["attn_out"] for r in res.results], axis=1)
    return f_full, a_full


if __name__ == "__main__":
    nc = build_core_program()
    print("program built ok")
